# revision 1
# baseline (speedup 1.0000x reference)
"""Trainium2 Bass kernel: multi-head self-attention (B=2, S=2048, D=1024, H=16).

Sharding: tensor-parallel over heads. Each of the 8 cores owns 2 heads
(128 of the 1024 hidden dims): Wq/Wk/Wv column-sharded, Wo row-sharded.
Each core computes a partial output Y_c = attn_c @ Wo_c; the host sums the
8 partials and adds bo.

Host-side prep: X is passed transposed (X^T [D, tokens]) so the kernel needs
no on-device transposes of the activations; weights are fed directly as f32r
(full-rate 4-byte matmul dtype on the PE).

Per-core pipeline:
  1. Q^T/K^T/V^T projections: out[d,s] += W[k,d].T @ X^T[k,s]  (f32r).
  2. V' blocks [ones64 | V_h] per 128-token key tile via PE transpose of V^T
     (the ones columns make the attnV matmul emit softmax denominators free).
  3. Flash attention per (batch, head): scores^T = K^T.T @ Q^T (K=64),
     exp on ScalarE (scale=1/8 fused; no max-subtraction needed: scores are
     ~N(0,1) so exp cannot overflow), attn^T accumulated over key tiles with
     V' stationary.
  4. Normalize: reciprocal of denominator rows, multiply fused with the
     PSUM->SBUF eviction of attn^T.
  5. Y partial = attn_cat^T.T @ Wo_c, DMA out.
"""

import sys

sys.path.insert(0, "/opt/trn_rl_repo")

import numpy as np

_LDW_PATCHED = False


def _patch_ldw_opt():
    """walrus's default --enable-ldw-opt=false makes every fused f32r matmul
    pay a full stationary-operand reload (~8x kernel slowdown measured).
    Rewrite the flag on the walrus command line. A marker tensor in the BIR
    (see _build_nc) keys the compile cache so stale ldw-opt=false NEFFs are
    never reused."""
    global _LDW_PATCHED
    if _LDW_PATCHED:
        return
    import concourse.bass_utils as BU

    orig_run = BU.run_command

    def patched_run(argv, **kw):
        argv = [
            a.replace("--enable-ldw-opt=false", "--enable-ldw-opt=true")
            if isinstance(a, str) else a
            for a in argv
        ]
        return orig_run(argv, **kw)

    BU.run_command = patched_run
    _LDW_PATCHED = True


B = 2
S = 2048
D = 1024
H = 16
HD = 64
NCORES = 8
DC = D // NCORES          # 128 head-dims per core (2 heads)
ST = B * S                # 4096 tokens total
NG = 8                    # projection s-groups
GS = ST // NG             # 512 tokens per group
SBLK = 1024               # attention query block
NT = S // 128             # 16 key tiles per batch

_CACHE = {}


def _legalize_waits(nc):
    """This walrus build accepts at most 1 sem wait per instruction
    (2 for EventSemaphore). Hoist excess waits onto same-engine NOPs."""
    from concourse import mybir

    ctr = 0
    for fn in nc.m.functions:
        for bb in fn.blocks:
            new = []
            for inst in bb.instructions:
                si = getattr(inst, "sync_info", None)
                waits = list(si.on_wait) if (si is not None and si.on_wait) else []
                cap = 2 if isinstance(inst, mybir.InstEventSemaphore) else 1
                if len(waits) > cap:
                    extra, keep = waits[:-cap], waits[-cap:]
                    for w in extra:
                        ctr += 1
                        nop = mybir.InstNoOp(
                            name=f"waitfix-{ctr}", ins=[], outs=[],
                            engine=inst.engine,
                        )
                        nop.sync_info = mybir.SyncInfo(on_wait=[w], on_update=[])
                        new.append(nop)
                    si.on_wait = keep
                new.append(inst)
            bb.instructions[:] = new
    return nc


def _build_nc(repeat=1, phases=('proj', 'attn', 'outp')):
    from contextlib import ExitStack

    import concourse.bass as bass
    import concourse.tile as tile
    from concourse import mybir
    from concourse.bass import ts
    from concourse.masks import make_identity

    f32 = mybir.dt.float32
    f32r = mybir.dt.float32r
    AF = mybir.ActivationFunctionType

    _patch_ldw_opt()
    nc = bass.Bass("TRN2", target_bir_lowering=False, debug=False)
    # Cache-key marker: BIR differs from any ldw-opt=false build.
    nc.dram_tensor("ldwopt_v3_marker", [1, 1], mybir.dt.float32, kind="Internal")
    xt_d = nc.dram_tensor("xt", [NG, 128, 8, GS], f32r, kind="ExternalInput").ap()
    wq_d = nc.dram_tensor("wq", [D, DC], f32r, kind="ExternalInput").ap()
    wk_d = nc.dram_tensor("wk", [D, DC], f32r, kind="ExternalInput").ap()
    wv_d = nc.dram_tensor("wv", [D, DC], f32r, kind="ExternalInput").ap()
    wo_d = nc.dram_tensor("wo", [DC, D], f32r, kind="ExternalInput").ap()
    bq_d = nc.dram_tensor("bq", [DC, 1], f32, kind="ExternalInput").ap()
    bk_d = nc.dram_tensor("bk", [DC, 1], f32, kind="ExternalInput").ap()
    bv_d = nc.dram_tensor("bv", [DC, 1], f32, kind="ExternalInput").ap()
    y_d = nc.dram_tensor("y", [ST, D], f32, kind="ExternalOutput").ap()

    with tile.TileContext(nc) as tc, ExitStack() as ctx:
        consts = ctx.enter_context(tc.tile_pool(name="consts", bufs=1))
        xtg_p = ctx.enter_context(tc.tile_pool(name="xtg", bufs=3))
        big_p = ctx.enter_context(tc.tile_pool(name="big", bufs=1))
        pt_p = ctx.enter_context(tc.tile_pool(name="pt", bufs=3))
        rc_p = ctx.enter_context(tc.tile_pool(name="rc", bufs=2))
        yst_p = ctx.enter_context(tc.tile_pool(name="yst", bufs=3))
        ps_p = ctx.enter_context(tc.tile_pool(name="ps", bufs=1, space="PSUM"))

        ident = consts.tile([128, 128], f32, name="ident")
        make_identity(nc, ident[:])

        # Weights with k on partitions: wq_sb[:, j, :] is the [k-chunk, d] lhsT.
        wq_sb = consts.tile([128, 8, 128], f32r, name="wq_sb")
        wk_sb = consts.tile([128, 8, 128], f32r, name="wk_sb")
        wv_sb = consts.tile([128, 8, 128], f32r, name="wv_sb")
        for wsb, wd in ((wq_sb, wq_d), (wk_sb, wk_d), (wv_sb, wv_d)):
            nc.sync.dma_start(
                wsb[:], wd.rearrange("(j p) d -> p j d", p=128)
            )
        wo_sb = consts.tile([128, D], f32r, name="wo_sb")
        nc.sync.dma_start(wo_sb[:], wo_d)
        bq_sb = consts.tile([128, 1], f32, name="bq_sb")
        bk_sb = consts.tile([128, 1], f32, name="bk_sb")
        bv_sb = consts.tile([128, 1], f32, name="bv_sb")
        for bsb, bd in ((bq_sb, bq_d), (bk_sb, bk_d), (bv_sb, bv_d)):
            nc.sync.dma_start(bsb[:], bd)

        qt = big_p.tile([128, ST], f32r, name="qt")
        kt = big_p.tile([128, ST], f32r, name="kt")
        vt = big_p.tile([128, ST], f32r, name="vt")
        # V': per (b, ti) a 256-col block [ones64 | V_A64 | ones64 | V_B64].
        vp = big_p.tile([128, B * NT * 256], f32r, name="vp")
        nc.gpsimd.memset(vp[:].bitcast(f32), 1.0)
        acat = [
            big_p.tile([128, S], f32r, name=f"acat{b}") for b in range(B)
        ]
        if "proj" not in phases and "attn" in phases:
            for t in (qt, kt):
                nc.gpsimd.memset(t[:].bitcast(f32), 0.0)
        if "attn" not in phases and "outp" in phases:
            for t in acat:
                nc.gpsimd.memset(t[:].bitcast(f32), 0.5)

        def proj_pair(gp):
            xtgs = []
            for g in (2 * gp, 2 * gp + 1):
                xtg = xtg_p.tile([128, 8, 512], f32r, name="xtg")
                nc.sync.dma_start(xtg[:], xt_d[g])
                xtgs.append(xtg)
            for wsb, bsb, out_t in (
                (wq_sb, bq_sb, qt), (wk_sb, bk_sb, kt), (wv_sb, bv_sb, vt)
            ):
                pj = ps_p.tile([128, 1024], f32, tag="ps1024", bufs=2, name="pj")
                for half in range(2):
                    for j in range(8):
                        nc.tensor.matmul(
                            pj[:, ts(half, 512)], wsb[:, j, :], xtgs[half][:, j, :],
                            start=(j == 0), stop=(j == 7),
                        )
                nc.vector.tensor_scalar_add(
                    out_t[:, ts(gp, 2 * GS)], pj[:], bsb[:]
                )

        def build_vp(b):
            for ti in range(NT):
                vps = ps_p.tile([128, 128], f32, tag="ps1024", bufs=2,
                                name="vps")
                nc.tensor.transpose(
                    vps[:],
                    vt[:, 2048 * b + 128 * ti: 2048 * b + 128 * (ti + 1)]
                    .bitcast(f32),
                    ident[:],
                )
                blk = 256 * (NT * b + ti)
                nc.vector.tensor_copy(
                    vp[:, blk + 64: blk + 128], vps[:, 0:64]
                )
                nc.vector.tensor_copy(
                    vp[:, blk + 192: blk + 256], vps[:, 64:128]
                )

        def attention(b, h):
            h0 = 64 * h
            for sb in range(2):
                s0 = 2048 * b + SBLK * sb
                att = ps_p.tile([128, SBLK], f32, tag="ps1024", bufs=2,
                                name="att")
                for tp in range(NT // 2):
                    sc = ps_p.tile([128, 2 * SBLK], f32, tag="ps2048", bufs=1,
                                   name="sc")
                    for half in range(2):
                        ti = 2 * tp + half
                        t0 = 2048 * b + 128 * ti
                        for ch in range(2):
                            nc.tensor.matmul(
                                sc[:, 1024 * half + 512 * ch:
                                   1024 * half + 512 * (ch + 1)],
                                kt[h0:h0 + 64, t0:t0 + 128],
                                qt[h0:h0 + 64,
                                   s0 + 512 * ch: s0 + 512 * (ch + 1)],
                                start=True, stop=True,
                            )
                    p = pt_p.tile([128, 2 * SBLK], f32r, name="pt")
                    nc.scalar.activation(p[:], sc[:], AF.Exp, scale=0.125)
                    for half in range(2):
                        ti = 2 * tp + half
                        blk = 256 * (NT * b + ti) + 128 * h
                        for ch in range(2):
                            nc.tensor.matmul(
                                att[:, ts(ch, 512)],
                                vp[:, blk: blk + 128],
                                p[:, 1024 * half + 512 * ch:
                                  1024 * half + 512 * (ch + 1)],
                                start=(ti == 0), stop=(ti == 15),
                            )
                rt = rc_p.tile([128, SBLK], f32, name="rt")
                nc.vector.reciprocal(rt[h0:h0 + 64, :], att[0:64, :])
                nc.vector.tensor_mul(
                    acat[b][h0:h0 + 64, SBLK * sb: SBLK * (sb + 1)],
                    rt[h0:h0 + 64, :],
                    att[64:128, :],
                )

        def outproj(b):
            for st in range(16):
                yp = ps_p.tile([128, D], f32, tag="ps1024", bufs=2, name="yp")
                for ch in range(2):
                    nc.tensor.matmul(
                        yp[:, ts(ch, 512)],
                        acat[b][:, ts(st, 128)],
                        wo_sb[:, ts(ch, 512)],
                        start=True, stop=True,
                    )
                ys = yst_p.tile([128, D], f32, name="ys")
                nc.vector.tensor_copy(ys[:], yp[:])
                nc.sync.dma_start(y_d[ts(16 * b + st, 128), :], ys[:])

        for _rep in range(repeat):
            if 'proj' in phases:
                for gp in range(2):
                    proj_pair(gp)
                build_vp(0)
            if 'attn' in phases:
                attention(0, 0)
                attention(0, 1)
            if 'proj' in phases:
                for gp in range(2, 4):
                    proj_pair(gp)
                build_vp(1)
            if 'outp' in phases:
                outproj(0)
            if 'attn' in phases:
                attention(1, 0)
                attention(1, 1)
            if 'outp' in phases:
                outproj(1)

    return _legalize_waits(nc)


def _get_nc(repeat=1, phases=('proj', 'attn', 'outp')):
    key = ("nc", repeat, phases)
    if key not in _CACHE:
        _CACHE[key] = _build_nc(repeat, phases)
    return _CACHE[key]


def _make_in_maps(inputs):
    x = np.asarray(inputs["inputs"], dtype=np.float32).reshape(ST, D)
    xt_flat = x.T  # [D, ST]
    # Pre-tile for the kernel's DMA layout: [g, p, j, s'] = XT[128j+p, 512g+s']
    xt = np.ascontiguousarray(
        xt_flat.reshape(8, 128, 8, GS).transpose(2, 1, 0, 3)
    )
    wq = np.asarray(inputs["Wq"], dtype=np.float32)
    wk = np.asarray(inputs["Wk"], dtype=np.float32)
    wv = np.asarray(inputs["Wv"], dtype=np.float32)
    wo = np.asarray(inputs["Wo"], dtype=np.float32)
    bq = np.asarray(inputs["bq"], dtype=np.float32)
    bk = np.asarray(inputs["bk"], dtype=np.float32)
    bv = np.asarray(inputs["bv"], dtype=np.float32)
    in_maps = []
    for c in range(NCORES):
        sl = slice(DC * c, DC * (c + 1))
        in_maps.append({
            "xt": xt,
            "wq": np.ascontiguousarray(wq[:, sl]),
            "wk": np.ascontiguousarray(wk[:, sl]),
            "wv": np.ascontiguousarray(wv[:, sl]),
            "wo": np.ascontiguousarray(wo[sl, :]),
            "bq": np.ascontiguousarray(bq[sl].reshape(DC, 1)),
            "bk": np.ascontiguousarray(bk[sl].reshape(DC, 1)),
            "bv": np.ascontiguousarray(bv[sl].reshape(DC, 1)),
        })
    return in_maps


def kernel(**inputs):
    from concourse.bass_utils import run_bass_kernel_spmd

    nc = _get_nc()
    in_maps = _make_in_maps(inputs)
    res = run_bass_kernel_spmd(nc, in_maps, core_ids=list(range(NCORES)))
    y = res.results[0]["y"].astype(np.float64)
    for c in range(1, NCORES):
        y += res.results[c]["y"]
    y += np.asarray(inputs["bo"], dtype=np.float64)
    return y.reshape(B, S, D).astype(np.float32)



# revision 19
# speedup vs baseline: 2.4375x; 2.4375x over previous
"""Trainium2 Bass kernel: multi-head self-attention (B=2, S=2048, D=1024, H=16).

Sharding: tensor-parallel over heads. Each of the 8 cores owns 2 heads
(128 of the 1024 hidden dims): Wq/Wk/Wv column-sharded, Wo row-sharded.
Each core computes a partial output Y_c = attn_c @ Wo_c; the host sums the
8 partials and adds bo + bv@Wo (softmax weights sum to 1, so the V bias
contributes bv@Wo to the output exactly; folding it host-side frees the
device V path of bias work).

v3 design (all bf16 outside PSUM; PE rate is 1 col/cycle either way):
  * V-direct projection: V tiles are computed token-major (stationary =
    X^T chunk, moving = Wv chunk) straight into PSUM and evicted into the
    attnV stationary layout vp = [ones64 | V64] per (batch, keytile, head).
    No feature-major vt, no PE transposes, no serial build_vp phase.
  * Globally software-pipelined attention: a continuous stream of
    (head, query-block, keytile) steps; attnV lags scores/exp by LAG steps
    so the in-order PE queue never waits on the scalar-engine exp. sc PSUM
    tiles are triple-buffered; att accumulators single-buffered with a
    fast DVE copy-evict (recip+mul then run from the SBUF copy).
  * outproj matmuls are folded INTO the attention stream (one 512-col
    matmul every other step) soaking up PE slack while Act is saturated
    with exp; only the last 8 output tiles form a small tail block.
  * PSUM: tag "sc" [128,1024]f32 x3 bufs (6 banks, shared by proj pj /
    scores sc / outproj yp / V tiles) + tag "att" x1 (2 banks) = 8 banks.
"""

import sys

sys.path.insert(0, "/opt/trn_rl_repo")

from collections import deque

import numpy as np

_LDW_PATCHED = False


def _patch_ldw_opt():
    """walrus's default --enable-ldw-opt=false makes every fused matmul
    pay a full stationary-operand reload (~8x kernel slowdown measured).
    Rewrite the flag on the walrus command line."""
    global _LDW_PATCHED
    if _LDW_PATCHED:
        return
    import concourse.bass_utils as BU

    orig_run = BU.run_command

    def patched_run(argv, **kw):
        argv = [
            a.replace("--enable-ldw-opt=false", "--enable-ldw-opt=true")
            if isinstance(a, str) else a
            for a in argv
        ]
        return orig_run(argv, **kw)

    BU.run_command = patched_run
    _LDW_PATCHED = True


B = 2
S = 2048
D = 1024
H = 16
HD = 64
NCORES = 8
DC = D // NCORES          # 128 head-dims per core (2 heads)
ST = B * S                # 4096 tokens total
NG = 8                    # projection s-groups
GS = ST // NG             # 512 tokens per group
SBLK = 1024               # attention query block
NT = S // 128             # 16 key tiles per batch
LAG = 2                   # attnV lags scores/exp by this many stream steps

_CACHE = {}


def _legalize_waits(nc):
    """This walrus build accepts at most 1 sem wait per instruction
    (2 for EventSemaphore). Hoist excess waits onto same-engine NOPs."""
    from concourse import mybir

    ctr = 0
    for fn in nc.m.functions:
        for bb in fn.blocks:
            new = []
            for inst in bb.instructions:
                si = getattr(inst, "sync_info", None)
                waits = list(si.on_wait) if (si is not None and si.on_wait) else []
                cap = 2 if isinstance(inst, mybir.InstEventSemaphore) else 1
                if len(waits) > cap:
                    extra, keep = waits[:-cap], waits[-cap:]
                    for w in extra:
                        ctr += 1
                        nop = mybir.InstNoOp(
                            name=f"waitfix-{ctr}", ins=[], outs=[],
                            engine=inst.engine,
                        )
                        nop.sync_info = mybir.SyncInfo(on_wait=[w], on_update=[])
                        new.append(nop)
                    si.on_wait = keep
                new.append(inst)
            bb.instructions[:] = new
    return nc


def _build_nc(repeat=1, phases=('proj', 'attn', 'outp')):
    from contextlib import ExitStack

    import concourse.bass as bass
    import concourse.tile as tile
    from concourse import mybir
    from concourse.bass import ts

    f32 = mybir.dt.float32
    bf16 = mybir.dt.bfloat16
    AF = mybir.ActivationFunctionType

    # ldw-opt=true (the old f32r baseline's patch) rejects bf16 ldweights in
    # this walrus build; run with the default ldw-opt=false instead.
    nc = bass.Bass("TRN2", target_bir_lowering=False, debug=False)
    nc.dram_tensor("bf16_noldw_v6_marker", [1, 1], mybir.dt.float32, kind="Internal")
    xt_d = nc.dram_tensor("xt", [NG, 128, 8, GS], bf16, kind="ExternalInput").ap()
    wq_d = nc.dram_tensor("wq", [D, DC], bf16, kind="ExternalInput").ap()
    wk_d = nc.dram_tensor("wk", [D, DC], bf16, kind="ExternalInput").ap()
    wv_d = nc.dram_tensor("wv", [D, DC], bf16, kind="ExternalInput").ap()
    wo_d = nc.dram_tensor("wo", [DC, D], bf16, kind="ExternalInput").ap()
    bq_d = nc.dram_tensor("bq", [DC, 1], f32, kind="ExternalInput").ap()
    bk_d = nc.dram_tensor("bk", [DC, 1], f32, kind="ExternalInput").ap()
    y_d = nc.dram_tensor("y", [ST, D], bf16, kind="ExternalOutput").ap()

    with tile.TileContext(nc) as tc, ExitStack() as ctx:
        consts = ctx.enter_context(tc.tile_pool(name="consts", bufs=1))
        xtg_p = ctx.enter_context(tc.tile_pool(name="xtg", bufs=3))
        big_p = ctx.enter_context(tc.tile_pool(name="big", bufs=1))
        pt_p = ctx.enter_context(tc.tile_pool(name="pt", bufs=4))
        atts_p = ctx.enter_context(tc.tile_pool(name="atts", bufs=2))
        rc_p = ctx.enter_context(tc.tile_pool(name="rc", bufs=2))
        yst_p = ctx.enter_context(tc.tile_pool(name="yst", bufs=3))
        ps_p = ctx.enter_context(tc.tile_pool(name="ps", bufs=1, space="PSUM"))

        # Weights with k on partitions: wq_sb[:, j, :] is the [k-chunk, d] lhsT.
        wq_sb = consts.tile([128, 8, 128], bf16, name="wq_sb")
        wk_sb = consts.tile([128, 8, 128], bf16, name="wk_sb")
        wv_sb = consts.tile([128, 8, 128], bf16, name="wv_sb")
        bq_sb = consts.tile([128, 1], f32, name="bq_sb")
        bk_sb = consts.tile([128, 1], f32, name="bk_sb")
        # Weights/biases ride the gpsimd SWDGE queue so they don't
        # serialize behind the (big) xt group loads on the sync queue.
        nc.gpsimd.dma_start(wq_sb[:], wq_d.rearrange("(j p) d -> p j d", p=128))
        nc.gpsimd.dma_start(bq_sb[:], bq_d)
        nc.gpsimd.dma_start(wk_sb[:], wk_d.rearrange("(j p) d -> p j d", p=128))
        nc.gpsimd.dma_start(bk_sb[:], bk_d)
        nc.gpsimd.dma_start(wv_sb[:], wv_d.rearrange("(j p) d -> p j d", p=128))
        wo_sb = consts.tile([128, D], bf16, name="wo_sb")
        nc.gpsimd.dma_start(wo_sb[:], wo_d)

        # Warm the Exp table while DMAs land (avoids 1.3us mid-kernel).
        wtmp = consts.tile([128, 1], bf16, name="wtmp")
        warm = consts.tile([128, 1], bf16, name="warm")
        nc.gpsimd.memset(wtmp[:], 0.0)
        nc.scalar.activation(warm[:], wtmp[:], AF.Exp)

        qt = big_p.tile([128, ST], bf16, name="qt")
        kt = big_p.tile([128, ST], bf16, name="kt")
        # vp[:, bi, h, :] attnV stationary: h=0 -> [V64 | ones64],
        # h=1 -> [ones64 | V64], so the V-dims of head h land on att
        # partitions 64h..64h+64 and the normalize mul is partition-aligned
        # (walrus rejects tensor_tensor with misaligned SBUF inputs).
        vp = big_p.tile([128, B * NT, 2, 128], bf16, name="vp")
        nc.gpsimd.memset(vp[:], 1.0)
        acat = [big_p.tile([128, S], bf16, name=f"acat{b}") for b in range(B)]
        if "proj" not in phases and "attn" in phases:
            for t in (qt, kt):
                nc.gpsimd.memset(t[:], 0.0)
        if "attn" not in phases and "outp" in phases:
            for t in acat:
                nc.gpsimd.memset(t[:], 0.5)

        def proj_pair(gp):
            """Groups 2gp, 2gp+1 (1024 tokens): Q,K feature-major into
            qt/kt; V token-major straight into vp blocks."""
            g, xtgs = projqk_gen(gp)
            for _ in g:
                pass
            proj_v(gp, xtgs)

        class FoldQueue:
            """Ordered generators interleaved into the attention stream,
            each pulled once per `pace` stream steps after `delay` steps."""

            def __init__(self):
                self.items = deque()

            def push(self, gen, pace=2, delay=0):
                if gen is not None:
                    self.items.append([gen, pace, delay, 0])

            def tick(self):
                if not self.items:
                    return
                it = self.items[0]
                gen, pace, delay, ctr = it
                it[3] = ctr + 1
                if ctr < delay or (ctr - delay) % pace != 0:
                    return
                try:
                    next(gen)
                except StopIteration:
                    self.items.popleft()
                    self.tick()

            def drain(self):
                while self.items:
                    for _ in self.items.popleft()[0]:
                        pass

        def projqk_gen(gp):
            """Q/K projection for group pair gp as a foldable generator
            (one 512-col matmul per next()); xtg DMAs are issued eagerly."""
            xtgs = []
            for g in (2 * gp, 2 * gp + 1):
                xtg = xtg_p.tile([128, 8, 512], bf16, name="xtg")
                nc.sync.dma_start(xtg[:], xt_d[g])
                xtgs.append(xtg)

            def run():
                for wsb, bsb, out_t in ((wq_sb, bq_sb, qt), (wk_sb, bk_sb, kt)):
                    pj = ps_p.tile([128, 1024], f32, tag="sc", bufs=3, name="pj")
                    for half in range(2):
                        for j in range(8):
                            nc.tensor.matmul(
                                pj[:, ts(half, 512)], wsb[:, j, :],
                                xtgs[half][:, j, :],
                                start=(j == 0), stop=(j == 7),
                            )
                            if not (half == 1 and j == 7):
                                yield
                    nc.vector.tensor_scalar_add(
                        out_t[:, ts(gp, 2 * GS)], pj[:], bsb[:]
                    )
                    yield

            return run(), xtgs

        def proj_v(gp, xtgs, evict_vec=False):
            """V token-major straight into vp blocks (bias folded host-side)."""
            for t in range(8):
                bi = 8 * gp + t
                half, tt = divmod(t, 4)
                vtile = ps_p.tile([128, 128], f32, tag="sc", bufs=3, name="vtile")
                for j in range(8):
                    nc.tensor.matmul(
                        vtile[:],
                        xtgs[half][:, j, ts(tt, 128)],
                        wv_sb[:, j, :],
                        start=(j == 0), stop=(j == 7),
                    )
                for hh in range(2):
                    dst = vp[:, bi, hh, 64 * hh: 64 * hh + 64]
                    if evict_vec:
                        nc.vector.tensor_copy(dst, vtile[:, ts(hh, 64)])
                    else:
                        nc.scalar.activation(dst, vtile[:, ts(hh, 64)], AF.Copy)

        def outproj_gen(b, st_lo, st_hi, evict_alt):
            """Yields once per 512-col matmul; evict+DMA ride the 2nd."""
            for k, st in enumerate(range(st_lo, st_hi)):
                yp = ps_p.tile([128, D], f32, tag="sc", bufs=3, name="yp")
                nc.tensor.matmul(
                    yp[:, 0:512], acat[b][:, ts(st, 128)], wo_sb[:, 0:512],
                    start=True, stop=True,
                )
                yield
                nc.tensor.matmul(
                    yp[:, 512:1024], acat[b][:, ts(st, 128)], wo_sb[:, 512:1024],
                    start=True, stop=True,
                )
                ys = yst_p.tile([128, D], bf16, name="ys")
                if evict_alt and k % 2 == 1:
                    nc.scalar.activation(ys[:], yp[:], AF.Copy)
                else:
                    nc.vector.tensor_copy(ys[:], yp[:])
                # Alternate y stores across two DGE queues so the tail flush
                # isn't serialized on one queue.
                if st % 2 == 0:
                    nc.sync.dma_start(y_d[ts(16 * b + st, 128), :], ys[:])
                else:
                    nc.gpsimd.dma_start(y_d[ts(16 * b + st, 128), :], ys[:])
                yield

        def _emit_attnv(ent):
            _, att, p, b, h, ti = ent
            blk = NT * b + ti
            for ch in range(2):
                nc.tensor.matmul(
                    att[:, ts(ch, 512)],
                    vp[:, blk, h, :],
                    p[:, ts(ch, 512)],
                    start=(ti == 0), stop=(ti == NT - 1),
                )

        def _emit_evict(ent):
            _, att, b, h, sb, last = ent
            h0 = 64 * h        # att rows of the V-dims (vp layout per head)
            d0 = 64 - h0       # att rows of the ones-trick denominators
            if last:
                # End of a batch stream: nothing waits on the att buffer, so
                # skip the staging copy and shorten the recip->mul chain.
                src = att
            else:
                atts = atts_p.tile([128, SBLK], f32, name="atts")
                nc.vector.tensor_copy(atts[:], att[:])
                src = atts
            rt = rc_p.tile([128, SBLK], f32, name="rt")
            nc.vector.reciprocal(rt[h0:h0 + 64, :], src[d0:d0 + 64, :])
            nc.vector.tensor_mul(
                acat[b][h0:h0 + 64, SBLK * sb: SBLK * (sb + 1)],
                rt[h0:h0 + 64, :],
                src[h0:h0 + 64, :],
            )

        def _pop(state, want_mm=True):
            """Emit pending attnV work: evictions freely, one matmul pair."""
            q = state["q"]
            while q:
                if q[0][0] == "ev":
                    _emit_evict(q.popleft())
                    continue
                if not want_mm:
                    return
                _emit_attnv(q.popleft())
                return

        def attn_block(b, h, sb, state, last=False):
            """16 keytile steps of one (head, query-block) unit, streamed
            through the shared pipeline state."""
            h0 = 64 * h
            s0 = 2048 * b + SBLK * sb
            att = ps_p.tile([128, SBLK], f32, tag="att", bufs=1, name="att")
            nmm = 0
            for ti in range(NT):
                t0 = 2048 * b + 128 * ti
                sc = ps_p.tile([128, SBLK], f32, tag="sc", bufs=3, name="sc")
                for ch in range(2):
                    nc.tensor.matmul(
                        sc[:, ts(ch, 512)],
                        kt[h0:h0 + 64, t0:t0 + 128],
                        qt[h0:h0 + 64, s0 + 512 * ch: s0 + 512 * (ch + 1)],
                        start=True, stop=True,
                    )
                p = pt_p.tile([128, SBLK], bf16, name="pt")
                nc.scalar.activation(p[:], sc[:], AF.Exp, scale=0.125)
                state["q"].append(("mm", att, p, b, h, ti))
                nmm += 1
                if nmm > LAG or state["primed"]:
                    state["primed"] = True
                    _pop(state)
                state["folds"].tick()
            state["q"].append(("ev", att, b, h, sb, last))

        def drain(state):
            q = state["q"]
            while q:
                if q[0][0] == "ev":
                    _emit_evict(q.popleft())
                else:
                    _emit_attnv(q.popleft())

        def drain_fold(fold):
            if fold is not None:
                for _ in fold:
                    pass

        def outproj_tail(b, st_lo, st_hi):
            g = outproj_gen(b, st_lo, st_hi, evict_alt=True)
            for _ in g:
                pass

        for _rep in range(repeat):
            do_p = "proj" in phases
            do_a = "attn" in phases
            do_o = "outp" in phases
            if do_p:
                proj_pair(0)
                proj_pair(1)
            xtgs2 = None
            if do_a:
                st0 = {"q": deque(), "primed": False, "folds": FoldQueue()}
                if do_p:
                    # Fold batch-1 Q/K projection into the batch-0 stream's
                    # PE slack (Act is the gate there).
                    g2, xtgs2 = projqk_gen(2)
                    st0["folds"].push(g2, pace=1, delay=6)
                attn_block(0, 0, 0, st0)
                attn_block(0, 1, 0, st0)
                if do_o:
                    st0["folds"].push(outproj_gen(0, 0, 8, evict_alt=False),
                                      pace=2, delay=5)
                attn_block(0, 0, 1, st0)
                attn_block(0, 1, 1, st0, last=True)
                drain(st0)
                st0["folds"].drain()
            elif do_o:
                outproj_tail(0, 0, 8)
            if do_p:
                if xtgs2 is None:
                    proj_pair(2)
                else:
                    proj_v(2, xtgs2)
                proj_pair(3)
            if do_a:
                st1 = {"q": deque(), "primed": False, "folds": FoldQueue()}
                if do_o:
                    st1["folds"].push(outproj_gen(0, 8, 16, evict_alt=False),
                                      pace=2, delay=0)
                attn_block(1, 0, 0, st1)
                attn_block(1, 1, 0, st1)
                if do_o:
                    st1["folds"].push(outproj_gen(1, 0, 8, evict_alt=False),
                                      pace=2, delay=5)
                attn_block(1, 0, 1, st1)
                attn_block(1, 1, 1, st1, last=True)
                drain(st1)
                st1["folds"].drain()
            elif do_o:
                outproj_tail(0, 8, 16)
                outproj_tail(1, 0, 8)
            if do_o:
                outproj_tail(1, 8, 16)

    return _legalize_waits(nc)


def _get_nc(repeat=1, phases=('proj', 'attn', 'outp')):
    key = ("nc", repeat, phases)
    if key not in _CACHE:
        _CACHE[key] = _build_nc(repeat, phases)
    return _CACHE[key]


def _make_in_maps(inputs):
    import ml_dtypes

    bf16 = ml_dtypes.bfloat16
    x = np.asarray(inputs["inputs"], dtype=np.float32).reshape(ST, D)
    xt_flat = x.T  # [D, ST]
    # Pre-tile for the kernel's DMA layout: [g, p, j, s'] = XT[128j+p, 512g+s']
    xt = np.ascontiguousarray(
        xt_flat.reshape(8, 128, 8, GS).transpose(2, 1, 0, 3)
    ).astype(bf16)
    wq = np.asarray(inputs["Wq"], dtype=np.float32).astype(bf16)
    wk = np.asarray(inputs["Wk"], dtype=np.float32).astype(bf16)
    wv = np.asarray(inputs["Wv"], dtype=np.float32).astype(bf16)
    wo = np.asarray(inputs["Wo"], dtype=np.float32).astype(bf16)
    bq = np.asarray(inputs["bq"], dtype=np.float32)
    bk = np.asarray(inputs["bk"], dtype=np.float32)
    in_maps = []
    for c in range(NCORES):
        sl = slice(DC * c, DC * (c + 1))
        in_maps.append({
            "xt": xt,
            "wq": np.ascontiguousarray(wq[:, sl]),
            "wk": np.ascontiguousarray(wk[:, sl]),
            "wv": np.ascontiguousarray(wv[:, sl]),
            "wo": np.ascontiguousarray(wo[sl, :]),
            "bq": np.ascontiguousarray(bq[sl].reshape(DC, 1)),
            "bk": np.ascontiguousarray(bk[sl].reshape(DC, 1)),
        })
    return in_maps


def kernel(**inputs):
    from concourse.bass_utils import run_bass_kernel_spmd

    nc = _get_nc()
    in_maps = _make_in_maps(inputs)
    res = run_bass_kernel_spmd(nc, in_maps, core_ids=list(range(NCORES)))
    y = res.results[0]["y"].astype(np.float64)
    for c in range(1, NCORES):
        y += res.results[c]["y"].astype(np.float64)
    # Softmax weights sum to 1, so the V bias contributes bv @ Wo exactly.
    bv = np.asarray(inputs["bv"], dtype=np.float64)
    wo_full = np.asarray(inputs["Wo"], dtype=np.float64)
    y += bv @ wo_full + np.asarray(inputs["bo"], dtype=np.float64)
    return y.reshape(B, S, D).astype(np.float32)
